# revision 51
# baseline (speedup 1.0000x reference)
"""Trainium2 Bass kernel for single-head causal attention.

  q = Xq @ Wq.T + bq ; k = Xk @ Wk.T + bk ; v = Xv @ Wv.T + bv
  out = softmax((q k^T + causal_mask)/sqrt(D)) @ v

Shapes: B=4, S=2048, D=1024, fp32 in/out.  8 NeuronCores, SPMD.

Sharding: core c handles batch b = c//2, parity h = c%2.  S splits into 16
q-tiles of 128; causal attention for q-tile g touches k-tiles 0..g.  Core
parity h owns q-tiles g = 2j + h (j = 0..7), and slot j statically
processes 2j+2 k-tiles on BOTH parities (identical SPMD program); the
h=0 core's last k-tile per slot is fully masked, so the per-core causal
mask is one static [128, 256] tile covering the last two k-tiles.

V-projection dedupe: the two cores of a batch pair each project only
their OWN half of V (8 s-tiles) and exchange halves through a pairwise
AllGather over DRAM bounce buffers (replica groups [2b, 2b+1]).  The
gathered buffer is rank-major == physical s-order, so v_sb reassembles
identically on both parities with a uniform SPMD program and the math
is bit-identical to projecting both halves locally.  The collective's
~55us doorbell-to-fill latency hides under the K/Q projections and the
scores phase.  (K-proj dedupe doesn't fit: K^T is consumed ~35us after
its half would finish bouncing, well inside the collective latency.)

Compute (all bf16 matmuls, f32 psum):
  - K^T, Q^T projected to [e-part, s] layout, V to [s-part, d]; all
    stay SBUF-resident except the V-half exchange roundtrip.
  - Scores are computed TRANSPOSED ([k, q] blocks): exp output feeds the
    P@V matmul directly as the stationary operand - no PE transposes.
  - softmax denominator = pe-block matmul against a ones column, giving
    [q-part, 1] psum, the right orientation for the final normalize
    (out = av * (1/den) + bv on DVE).
  - attention is software-pipelined one slot deep: scores(j+1) are issued
    before P@V(j) so the exp never stalls the tensor engine.
"""

from contextlib import ExitStack

import ml_dtypes
import numpy as np

import concourse.bacc as bacc
import concourse.mybir as mybir
import concourse.tile as tile
from concourse.bass_utils import run_bass_kernel_spmd

P = 128
D = 1024
S = 2048
B = 4
N_CORES = 8
EO = D // P            # 8 contraction chunks of 128
DO = D // P            # 8 output-dim chunks of 128
NT = S // P            # 16 k/s tiles of 128
NQ = 8                 # q-tile slots per core
F32 = mybir.dt.float32
BF16 = mybir.dt.bfloat16
FP8 = mybir.dt.float8e4
NEG = -1.0e9
BF = ml_dtypes.bfloat16

_PROG_CACHE = {}


def _slot_gtiles(h, causal):
    """q-tile ids (units of 128 rows) owned by parity-h core, slot order."""
    if causal:
        return [2 * j + h for j in range(NQ)]
    return [8 * h + j for j in range(NQ)]


def build_program(causal: bool):
    nc = bacc.Bacc(trn_type="TRN2", target_bir_lowering=False, debug=False)

    def din(name, shape, dt=BF16):
        return nc.dram_tensor(name, shape, dt, kind="ExternalInput").ap()

    xq = din("xq", [P, EO, 1024], FP8)   # Xq^T for this core's 8 q-tiles
    # Xk^T own half only (2 chunks of 512 k-rows): K is deduped across the
    # batch pair the same way as V, via an early pairwise AllGather
    xk = din("xk", [2, P, EO, 512], FP8)
    # Xv^T for this core's OWN half of the sequence (8 s-tiles): the two
    # cores of a batch pair each project half of V and exchange the halves
    # via a pairwise AllGather (bit-identical to projecting both halves).
    NVO = NT // 2
    xv = din("xv", [NVO, P, EO, P])      # Xv^T own half, blocked [s-tile][e][s]
    wq = din("wq", [P, EO, D], FP8)
    wk = din("wk", [P, EO, D], FP8)
    wv = din("wv", [P, EO, D])
    bq = din("bq", [P, DO], F32)
    bk = din("bk", [P, DO], F32)
    bv = din("bv", [P, D], F32)
    msk = din("msk", [P, 2 * P], F32)    # causal mask for last 2 k-tiles
    out = nc.dram_tensor("out", [NQ, P, D], F32, kind="ExternalOutput").ap()

    Ident = mybir.ActivationFunctionType.Identity
    Copy = mybir.ActivationFunctionType.Copy
    Exp = mybir.ActivationFunctionType.Exp
    add = mybir.AluOpType.add
    mult = mybir.AluOpType.mult

    # slot j processes nkt[j] k-tiles - identical on every core
    nkt = [2 * j + 2 if causal else NT for j in range(NQ)]

    with tile.TileContext(nc, pool_alloc_mode="queue") as tc, ExitStack() as top:
        const = top.enter_context(tc.tile_pool(name="const", bufs=1))
        bq_sb = const.tile([P, DO], F32)
        bk_sb = const.tile([P, DO], F32)
        bv_sb = const.tile([P, D], F32)
        msk_sb = const.tile([P, 2 * P], F32)
        ones_sb = const.tile([P, 1], BF16)
        nc.gpsimd.memset(ones_sb, 1.0)




        # resident projected tensors
        res = top.enter_context(tc.tile_pool(name="res", bufs=1))
        kt_sb = res.tile([P, DO, S], FP8, name="kt_sb")      # K^T [e, k]
        qt_sb = res.tile([P, DO, 1024], FP8, name="qt_sb")   # Q^T [e, q]
        v_sb = res.tile([P, NT, D], BF16, name="v_sb")       # V [s, d] blocked

        # DRAM bounce buffers for the pairwise V-half AllGather, split into
        # two 1MB collectives: the first (own s-tiles 0-3) fires ~14us
        # earlier than a single 2MB gather would complete, halving the
        # time-to-first-V-data and the exposure to collective-latency noise
        ccp = top.enter_context(tc.tile_pool(name="ccp", bufs=1, space="DRAM"))
        NVC = NVO // 2
        vins = [ccp.tile([P, NVC, D], BF16, name=f"vin{i}") for i in range(2)]
        vouts = [ccp.tile([2, P, NVC, D], BF16, name=f"vout{i}")
                 for i in range(2)]
        kin = ccp.tile([P, DO, S // 2], FP8, name="kin")
        kout = ccp.tile([2, P, DO, S // 2], FP8, name="kout")

        # ---------------- projections ----------------
        with tc.tile_pool(name="wt", bufs=2) as wtp, \
             tc.tile_pool(name="xin", bufs=4) as xinp, \
             tc.tile_pool(name="xqp", bufs=1) as xqp, \
             tc.tile_pool(name="xvp", bufs=8) as xvp, \
             tc.tile_pool(name="psA", bufs=3, space="PSUM") as psA, \
             tc.tile_pool(name="psB", bufs=2, space="PSUM") as psB:

            # K-own projection FIRST: it is the smallest phase whose output
            # feeds a collective, so its gather gets the longest runway
            # before the scores phase consumes kt_sb.  wk/xk-own get top
            # DMA priority; wv/xv stream in during the K phase.
            wk_sb = wtp.tile([P, EO, D], FP8, tag="wt", name="wk_sb")
            xk_ts = []
            nc.scalar.dma_start(out=bk_sb, in_=bk)  # needed at 1st K evict
            for e2 in range(0, EO, 2):
                (nc.sync if e2 % 4 == 0 else nc.gpsimd).dma_start(
                    out=wk_sb[:, e2:e2 + 2, :], in_=wk[:, e2:e2 + 2, :])
            for kc in range(2):
                xk_t = xinp.tile([P, EO, 512], FP8, tag="xin", name=f"xk_t{kc}")
                xk_ts.append(xk_t)
                (nc.sync if kc == 0 else nc.gpsimd).dma_start(out=xk_t,
                                                             in_=xk[kc])
            # V inputs next (consumed from ~28us)
            wv_sb = wtp.tile([P, EO, D], BF16, tag="wt", name="wv_sb")
            xv_t0 = xvp.tile([P, EO, P], BF16, tag="xv", name="xv_t0")
            qs = [nc.sync, nc.gpsimd]
            for eo in range(EO):
                qs[eo % 2].dma_start(out=wv_sb[:, eo, :], in_=wv[:, eo, :])
                qs[(eo + 1) % 2].dma_start(out=xv_t0[:, eo, :],
                                           in_=xv[0, :, eo, :])
            xv_ts = [xv_t0]
            for st in range(1, NVO):
                xv_t = xvp.tile([P, EO, P], BF16, tag="xv", name=f"xv_t{st}")
                xv_ts.append(xv_t)
                qs[st % 2].dma_start(out=xv_t, in_=xv[st])
            wq_sb = wtp.tile([P, EO, D], FP8, tag="wt", name="wq_sb")
            xq_t = xqp.tile([P, EO, 1024], FP8, name="xq_t")

            # K-own -> k_own staging (bias folded in, fp8 DoubleRow), then
            # bounce per-kc so the first half flies while the second projects
            k_own = xinp.tile([P, DO, 1024], FP8, tag="kown", bufs=1,
                              name="k_own")
            for kc in range(2):
                xk_t = xk_ts[kc]
                for do in range(DO):
                    ps = psA.tile([P, 512], F32, tag="psA", name=f"psk{kc}_{do}")
                    for m in range(4):
                        nc.tensor.matmul(
                            ps,
                            lhsT=wk_sb[:, 2 * m:2 * m + 2, do * P:(do + 1) * P],
                            rhs=xk_t[:, 2 * m:2 * m + 2, :],
                            start=(m == 0), stop=(m == 3),
                            perf_mode=mybir.MatmulPerfMode.DoubleRow)
                    nc.scalar.activation(
                        out=k_own[:, do, kc * 512:(kc + 1) * 512], in_=ps,
                        func=Ident, bias=bk_sb[:, do:do + 1], scale=1.0 / 16)
            for kc in range(2):
                nc.sync.dma_start(out=kin[:, :, kc * 512:(kc + 1) * 512],
                                  in_=k_own[:, :, kc * 512:(kc + 1) * 512])
            groups = [[2 * p, 2 * p + 1] for p in range(N_CORES // 2)]
            nc.gpsimd.collective_compute(
                "AllGather", mybir.AluOpType.bypass, replica_groups=groups,
                ins=[kin.opt()], outs=[kout.opt()])
            # K fill: kt_sb is written ONLY by these (gpsimd-only rule:
            # gpsimd issues no later non-cc DMAs, so a late collective can't
            # FIFO-block the attention-phase output DMAs on sync/scalar)
            for r in range(2):
                nc.gpsimd.dma_start(out=kt_sb[:, :, r * 1024:(r + 1) * 1024],
                                    in_=kout[r])

            # V-own projection (evict on DVE; scalar is busy with K evicts)
            v_own = xvp.tile([P, NVO, D], BF16, tag="vown", bufs=1,
                             name="v_own")
            for st in range(NVO):
                xv_t = xv_ts[st]
                ps2 = psB.tile([P, D], F32, tag="psB", name=f"psv{st}")
                # eo-outer: each xv stationary block is loaded once and used
                # for both output halves (the two psum groups are in
                # different banks, so interleaving them is safe)
                for eo in range(EO):
                    for half in range(2):
                        nc.tensor.matmul(
                            ps2[:, half * 512:(half + 1) * 512],
                            lhsT=xv_t[:, eo, :],
                            rhs=wv_sb[:, eo, half * 512:(half + 1) * 512],
                            start=(eo == 0), stop=(eo == EO - 1))
                nc.vector.tensor_copy(out=v_own[:, st, :], in_=ps2)

            # wq/xq (2MB, not consumed until the Q projection) are gated
            # behind the 4th V eviction to keep them out of the window
            # where the V bounces fight for DMA-ring slots; the remaining
            # consts ride behind them (needed later still)
            gate2_sb = const.tile([P, 1], BF16)
            nc.scalar.activation(out=gate2_sb, in_=v_own[:, 3, :1], func=Copy)
            nc.scalar.dma_start(out=wq_sb, in_=wq)
            nc.scalar.dma_start(out=xq_t, in_=xq)
            nc.scalar.dma_start(out=bq_sb, in_=bq)
            nc.scalar.dma_start(out=msk_sb, in_=msk)
            nc.scalar.dma_start(out=bv_sb, in_=bv)

            # V bounces + two half-gathers (rank-major == physical s-order,
            # so v_sb reassembles identically on both cores); fills gpsimd
            for i in range(2):
                for st in range(NVC):
                    nc.sync.dma_start(out=vins[i][:, st, :],
                                      in_=v_own[:, i * NVC + st, :])
                nc.gpsimd.collective_compute(
                    "AllGather", mybir.AluOpType.bypass, replica_groups=groups,
                    ins=[vins[i].opt()], outs=[vouts[i].opt()])
            for i in range(2):
                for r in range(2):
                    nc.gpsimd.dma_start(
                        out=v_sb[:, r * NVO + i * NVC:r * NVO + (i + 1) * NVC, :],
                        in_=vouts[i][r])

            # Q projection -> qt_sb [e-part, q], bias folded in (fp8 DoubleRow)
            for sc in range(2):
                for do in range(DO):
                    ps = psA.tile([P, 512], F32, tag="psA", name=f"psq{sc}_{do}")
                    for m in range(4):
                        nc.tensor.matmul(
                            ps,
                            lhsT=wq_sb[:, 2 * m:2 * m + 2, do * P:(do + 1) * P],
                            rhs=xq_t[:, 2 * m:2 * m + 2, sc * 512:(sc + 1) * 512],
                            start=(m == 0), stop=(m == 3),
                            perf_mode=mybir.MatmulPerfMode.DoubleRow)
                    nc.scalar.activation(
                        out=qt_sb[:, do, sc * 512:(sc + 1) * 512], in_=ps,
                        func=Ident, bias=bq_sb[:, do:do + 1], scale=1.0 / 16)

        # ---------------- attention ----------------
        # scores k-chunk-major: one stationary K-tile streams against ALL
        # active slots' Q columns (slots active for chunk c are j >= 2c,
        # contiguous in qt) - amortizes the fp8 weight loads.  exp lands in
        # pe_all[k-tile][slot*128], then P@V runs slot-major as before.
        with tc.tile_pool(name="pep", bufs=1) as pep, \
             tc.tile_pool(name="recp", bufs=2) as recp, \
             tc.tile_pool(name="outp", bufs=3) as outp, \
             tc.tile_pool(name="psS", bufs=3, space="PSUM") as psS, \
             tc.tile_pool(name="psV", bufs=2, space="PSUM") as psV, \
             tc.tile_pool(name="psD", bufs=1, space="PSUM") as psD:

            pe_all = pep.tile([P, NT, 1024], BF16, name="pe_all")
            dn_ps = psD.tile([P, NQ], F32, tag="dn", name="dn_ps")

            for c in range(4):
                j0 = 2 * c if causal else 0
                w = (NQ - j0) * P
                for i in range(4):
                    t = 4 * c + i
                    dcol = (t // 2 - j0) * P        # diagonal slot's column
                    for p0 in range(0, w, 512):
                        wp = min(512, w - p0)
                        ps = psS.tile([P, wp], F32, tag="s", name=f"ps{t}_{p0}")
                        for m in range(4):
                            nc.tensor.matmul(
                                ps,
                                lhsT=kt_sb[:, 2 * m:2 * m + 2, t * P:(t + 1) * P],
                                rhs=qt_sb[:, 2 * m:2 * m + 2,
                                          j0 * P + p0:j0 * P + p0 + wp],
                                start=(m == 0), stop=(m == 3),
                                perf_mode=mybir.MatmulPerfMode.DoubleRow)
                        if causal and p0 <= dcol < p0 + wp:
                            nc.vector.tensor_tensor(
                                out=ps[:, dcol - p0:dcol - p0 + P],
                                in0=ps[:, dcol - p0:dcol - p0 + P],
                                in1=msk_sb[:, (t % 2) * P:(t % 2 + 1) * P],
                                op=add)
                        nc.scalar.activation(
                            out=pe_all[:, t, j0 * P + p0:j0 * P + p0 + wp],
                            in_=ps, func=Exp, scale=float(1.0 / np.sqrt(D)))

            def av_slot(j):
                n_t = nkt[j]
                av = psV.tile([P, D], F32, tag="av", name=f"av{j}")
                for t in range(n_t):
                    pblk = pe_all[:, t, j * P:(j + 1) * P]
                    nc.tensor.matmul(
                        dn_ps[:, j:j + 1], lhsT=pblk, rhs=ones_sb,
                        start=(t == 0), stop=(t == n_t - 1))
                    for half in range(2):
                        nc.tensor.matmul(
                            av[:, half * 512:(half + 1) * 512],
                            lhsT=pblk,
                            rhs=v_sb[:, t, half * 512:(half + 1) * 512],
                            start=(t == 0), stop=(t == n_t - 1))
                rec = recp.tile([P, 1], F32, tag="rec", name=f"rec{j}")
                nc.vector.reciprocal(out=rec, in_=dn_ps[:, j:j + 1])
                o = outp.tile([P, D], F32, tag="o", name=f"o{j}")
                last = j == (0 if causal else NQ - 1)
                nhf = 4 if last else 2
                hw = D // nhf
                for hf in range(nhf):
                    nc.vector.scalar_tensor_tensor(
                        out=o[:, hf * hw:(hf + 1) * hw],
                        in0=av[:, hf * hw:(hf + 1) * hw], scalar=rec,
                        in1=bv_sb[:, hf * hw:(hf + 1) * hw],
                        op0=mult, op1=add)
                    eng = nc.sync if hf % 2 == 0 else nc.scalar
                    eng.dma_start(out=out[j, :, hf * hw:(hf + 1) * hw],
                                  in_=o[:, hf * hw:(hf + 1) * hw])

            # ascending so early AVs only need early exps and the tail is
            # a single slot's drain chain (rec -> normalize -> out DMA)
            for j in range(NQ):
                av_slot(j)

    nc.compile()
    return nc


def _get_program(causal: bool):
    key = bool(causal)
    if key not in _PROG_CACHE:
        _PROG_CACHE[key] = build_program(key)
    return _PROG_CACHE[key]


def _shard_inputs(encoded_q, encoded_k, encoded_v, W_q, b_q, W_k, b_k,
                  W_v, b_v, causal):
    """Build the per-core in_maps (all host-side numpy, bf16 payloads)."""
    F8 = mybir.dt.np(FP8)
    wqh = np.ascontiguousarray(
        (16.0 * W_q.T).reshape(EO, P, D).transpose(1, 0, 2)).astype(F8)
    wkh = np.ascontiguousarray(
        (16.0 * W_k.T).reshape(EO, P, D).transpose(1, 0, 2)).astype(F8)
    wvh = np.ascontiguousarray(
        W_v.T.reshape(EO, P, D).transpose(1, 0, 2)).astype(BF)
    bqh = np.ascontiguousarray(b_q.reshape(DO, P).T)
    bkh = np.ascontiguousarray(b_k.reshape(DO, P).T)
    bvh = np.ascontiguousarray(np.broadcast_to(b_v, (P, D)))

    ki = np.arange(P)[:, None]
    qi = np.arange(P)[None, :]
    tri = np.where(ki <= qi, 0.0, NEG).astype(np.float32)   # diagonal block
    zer = np.zeros((P, P), np.float32)
    ninf = np.full((P, P), NEG, np.float32)
    # h=0: slot j owns g=2j -> k-tile 2j is diagonal, 2j+1 fully masked
    # h=1: slot j owns g=2j+1 -> k-tile 2j unmasked, 2j+1 diagonal
    mskh = [np.concatenate([tri, ninf], 1), np.concatenate([zer, tri], 1)]

    in_maps = []
    for c in range(N_CORES):
        b, h = divmod(c, 2)
        gts = _slot_gtiles(h, causal)
        Xq = np.concatenate([encoded_q[b, g * P:(g + 1) * P, :] for g in gts], 0)
        xqh = np.ascontiguousarray(
            Xq.T.reshape(EO, P, 1024).transpose(1, 0, 2)).astype(F8)
        # own K half only: parity-h core projects k rows [h*S/2, (h+1)*S/2)
        xkh = np.ascontiguousarray(
            encoded_k[b, h * (S // 2):(h + 1) * (S // 2), :].T
            .reshape(EO, P, 2, 512).transpose(2, 1, 0, 3)).astype(F8)
        # own V half only: parity-h core projects s rows [h*S/2, (h+1)*S/2)
        xvh = np.ascontiguousarray(
            encoded_v[b, h * (S // 2):(h + 1) * (S // 2), :].T
            .reshape(EO, P, NT // 2, P).transpose(2, 1, 0, 3)).astype(BF)
        in_maps.append({
            "xq": xqh, "xk": xkh, "xv": xvh,
            "wq": wqh, "wk": wkh, "wv": wvh,
            "bq": bqh, "bk": bkh, "bv": bvh,
            "msk": mskh[h] if causal else np.zeros((P, 2 * P), np.float32),
        })
    return in_maps


def kernel(encoded_q, encoded_k, encoded_v, W_q, b_q, W_k, b_k, W_v, b_v,
           parameter_mask, _want_trace=False, _trace_dir=None):
    causal = bool(np.asarray(parameter_mask).item())
    encoded_q = np.asarray(encoded_q, np.float32)
    encoded_k = np.asarray(encoded_k, np.float32)
    encoded_v = np.asarray(encoded_v, np.float32)
    nc = _get_program(causal)
    in_maps = _shard_inputs(encoded_q, encoded_k, encoded_v,
                            np.asarray(W_q, np.float32), np.asarray(b_q, np.float32),
                            np.asarray(W_k, np.float32), np.asarray(b_k, np.float32),
                            np.asarray(W_v, np.float32), np.asarray(b_v, np.float32),
                            causal)
    kw = {}
    if _want_trace:
        kw = dict(trace=True, tmpdir=_trace_dir)
    elif not _PROG_CACHE.get(("warm", causal)):
        # first execution pays collective-communicator init (~hundreds of us
        # of skew on one core); absorb it in a throwaway run
        run_bass_kernel_spmd(nc, in_maps, core_ids=list(range(N_CORES)))
        _PROG_CACHE[("warm", causal)] = True
    res = run_bass_kernel_spmd(nc, in_maps, core_ids=list(range(N_CORES)), **kw)

    full = np.empty((B, S, D), np.float32)
    for c in range(N_CORES):
        b, h = divmod(c, 2)
        o = res.results[c]["out"]
        for j, g in enumerate(_slot_gtiles(h, causal)):
            full[b, g * P:(g + 1) * P, :] = o[j]
    if _want_trace:
        return full, res
    return full



# revision 52
# speedup vs baseline: 1.1635x; 1.1635x over previous
"""Trainium2 Bass kernel for single-head causal attention.

  q = Xq @ Wq.T + bq ; k = Xk @ Wk.T + bk ; v = Xv @ Wv.T + bv
  out = softmax((q k^T + causal_mask)/sqrt(D)) @ v

Shapes: B=4, S=2048, D=1024, fp32 in/out.  8 NeuronCores, SPMD.

Sharding: core c handles batch b = c//2, parity h = c%2.  S splits into 16
q-tiles of 128; causal attention for q-tile g touches k-tiles 0..g.  Core
parity h owns q-tiles g = 2j + h (j = 0..7), and slot j statically
processes 2j+2 k-tiles on BOTH parities (identical SPMD program); the
h=0 core's last k-tile per slot is fully masked, so the per-core causal
mask is one static [128, 256] tile covering the last two k-tiles.

V-projection dedupe: the two cores of a batch pair each project only
their OWN half of V (8 s-tiles) and exchange halves through a pairwise
AllGather over DRAM bounce buffers (replica groups [2b, 2b+1]).  The
gathered buffer is rank-major == physical s-order, so v_sb reassembles
identically on both parities with a uniform SPMD program and the math
is bit-identical to projecting both halves locally.  The collective's
~55us doorbell-to-fill latency hides under the K/Q projections and the
scores phase.  (K-proj dedupe doesn't fit: K^T is consumed ~35us after
its half would finish bouncing, well inside the collective latency.)

Compute (all bf16 matmuls, f32 psum):
  - K^T, Q^T projected to [e-part, s] layout, V to [s-part, d]; all
    stay SBUF-resident except the V-half exchange roundtrip.
  - Scores are computed TRANSPOSED ([k, q] blocks): exp output feeds the
    P@V matmul directly as the stationary operand - no PE transposes.
  - softmax denominator = pe-block matmul against a ones column, giving
    [q-part, 1] psum, the right orientation for the final normalize
    (out = av * (1/den) + bv on DVE).
  - attention is software-pipelined one slot deep: scores(j+1) are issued
    before P@V(j) so the exp never stalls the tensor engine.
"""

from contextlib import ExitStack

import ml_dtypes
import numpy as np

import concourse.bacc as bacc
import concourse.mybir as mybir
import concourse.tile as tile
from concourse.bass_utils import run_bass_kernel_spmd

P = 128
D = 1024
S = 2048
B = 4
N_CORES = 8
EO = D // P            # 8 contraction chunks of 128
DO = D // P            # 8 output-dim chunks of 128
NT = S // P            # 16 k/s tiles of 128
NQ = 8                 # q-tile slots per core
F32 = mybir.dt.float32
BF16 = mybir.dt.bfloat16
FP8 = mybir.dt.float8e4
NEG = -1.0e9
BF = ml_dtypes.bfloat16

_PROG_CACHE = {}


def _slot_gtiles(h, causal):
    """q-tile ids (units of 128 rows) owned by parity-h core, slot order."""
    if causal:
        return [2 * j + h for j in range(NQ)]
    return [8 * h + j for j in range(NQ)]


def build_program(causal: bool):
    nc = bacc.Bacc(trn_type="TRN2", target_bir_lowering=False, debug=False)

    def din(name, shape, dt=BF16):
        return nc.dram_tensor(name, shape, dt, kind="ExternalInput").ap()

    xq = din("xq", [P, EO, 1024], FP8)   # Xq^T for this core's 8 q-tiles
    xk = din("xk", [4, P, EO, 512], FP8)  # Xk^T, chunked along s
    # Xv^T for this core's OWN half of the sequence (8 s-tiles): the two
    # cores of a batch pair each project half of V and exchange the halves
    # via a pairwise AllGather (bit-identical to projecting both halves).
    NVO = NT // 2
    xv = din("xv", [NVO, P, EO, P])      # Xv^T own half, blocked [s-tile][e][s]
    wq = din("wq", [P, EO, D], FP8)
    wk = din("wk", [P, EO, D], FP8)
    wv = din("wv", [P, EO, D])
    bq = din("bq", [P, DO], F32)
    bk = din("bk", [P, DO], F32)
    bv = din("bv", [P, D], F32)
    msk = din("msk", [P, 2 * P], F32)    # causal mask for last 2 k-tiles
    out = nc.dram_tensor("out", [NQ, P, D], F32, kind="ExternalOutput").ap()

    Ident = mybir.ActivationFunctionType.Identity
    Copy = mybir.ActivationFunctionType.Copy
    Exp = mybir.ActivationFunctionType.Exp
    add = mybir.AluOpType.add
    mult = mybir.AluOpType.mult

    # slot j processes nkt[j] k-tiles - identical on every core
    nkt = [2 * j + 2 if causal else NT for j in range(NQ)]

    with tile.TileContext(nc, pool_alloc_mode="queue") as tc, ExitStack() as top:
        const = top.enter_context(tc.tile_pool(name="const", bufs=1))
        bq_sb = const.tile([P, DO], F32)
        bk_sb = const.tile([P, DO], F32)
        bv_sb = const.tile([P, D], F32)
        msk_sb = const.tile([P, 2 * P], F32)
        ones_sb = const.tile([P, 1], BF16)
        nc.gpsimd.memset(ones_sb, 1.0)




        # resident projected tensors
        res = top.enter_context(tc.tile_pool(name="res", bufs=1))
        kt_sb = res.tile([P, DO, S], FP8, name="kt_sb")      # K^T [e, k]
        qt_sb = res.tile([P, DO, 1024], FP8, name="qt_sb")   # Q^T [e, q]
        v_sb = res.tile([P, NT, D], BF16, name="v_sb")       # V [s, d] blocked

        # DRAM bounce buffers for the pairwise V-half AllGather, split into
        # two 1MB collectives: the first (own s-tiles 0-3) fires ~14us
        # earlier than a single 2MB gather would complete, halving the
        # time-to-first-V-data and the exposure to collective-latency noise
        ccp = top.enter_context(tc.tile_pool(name="ccp", bufs=1, space="DRAM"))
        NVC = NVO // 2
        vins = [ccp.tile([P, NVC, D], BF16, name=f"vin{i}") for i in range(2)]
        vouts = [ccp.tile([2, P, NVC, D], BF16, name=f"vout{i}")
                 for i in range(2)]

        # ---------------- projections ----------------
        with tc.tile_pool(name="wt", bufs=2) as wtp, \
             tc.tile_pool(name="xin", bufs=4) as xinp, \
             tc.tile_pool(name="xqp", bufs=1) as xqp, \
             tc.tile_pool(name="xvp", bufs=6) as xvp, \
             tc.tile_pool(name="psA", bufs=3, space="PSUM") as psA, \
             tc.tile_pool(name="psB", bufs=2, space="PSUM") as psB:

            # V projection FIRST: bf16 (1 cyc/row) gives the fp8 K/Q
            # inputs the whole phase to land, so the (2x faster, DMA-hungry)
            # DoubleRow projections never starve.  The startup-critical first
            # chunks (wv/xv eo=0) get their own small descriptors so the
            # first matmul isn't gated on a megabyte landing.
            # Queue discipline: big blocking K/Q input issues live ONLY on
            # scalar (whose first compute need - the K evictions - comes
            # late); sync and gpsimd carry the V-phase inputs + bounces so
            # nothing FIFO-blocks the eviction->bounce->collective chain.
            # The collective can only fire once the WHOLE own-V half is
            # projected, and that needs all of wv - so wv/xv get absolute
            # DMA priority on ALL THREE queues; the K/Q inputs queue up
            # behind them (their compute starts ~25us later).
            wv_sb = wtp.tile([P, EO, D], BF16, tag="wt", name="wv_sb")
            xv_t0 = xvp.tile([P, EO, P], BF16, tag="xv", name="xv_t0")
            qs = [nc.sync, nc.gpsimd, nc.scalar]
            for eo in range(EO):
                qs[eo % 3].dma_start(out=wv_sb[:, eo, :], in_=wv[:, eo, :])
                qs[(eo + 1) % 3].dma_start(out=xv_t0[:, eo, :], in_=xv[0, :, eo, :])
            # consts are needed only at the first K-proj eviction (~45us);
            # park them on scalar behind the K/Q inputs so sync/gpsimd can
            # keep feeding the V phase
            nc.scalar.dma_start(out=bk_sb, in_=bk)
            nc.scalar.dma_start(out=bq_sb, in_=bq)
            nc.scalar.dma_start(out=msk_sb, in_=msk)
            nc.scalar.dma_start(out=bv_sb, in_=bv)
            # Gate the K/Q input wave behind the V working set's arrival:
            # this tiny copy reads the last xv_t0 chunk, so the big scalar
            # DMAs behind it in FIFO order can't steal HBM bandwidth from
            # wv/xv during the V phase's startup-critical first ~15us.
            gate_sb = const.tile([P, 1], BF16)
            nc.scalar.activation(out=gate_sb, in_=xv_t0[:, 7, :1], func=Copy)
            # K/Q inputs: single big transfers on scalar (whose first compute
            # need - the K evictions - comes only after the V phase)
            wk_sb = wtp.tile([P, EO, D], FP8, tag="wt", name="wk_sb")
            nc.scalar.dma_start(out=wk_sb, in_=wk)
            xk_ts = []
            for kc in range(4):
                xk_t = xinp.tile([P, EO, 512], FP8, tag="xin", name=f"xk_t{kc}")
                xk_ts.append(xk_t)
                nc.scalar.dma_start(out=xk_t, in_=xk[kc])
            wq_sb = wtp.tile([P, EO, D], FP8, tag="wt", name="wq_sb")
            xq_t = xqp.tile([P, EO, 1024], FP8, name="xq_t")

            v_own = xvp.tile([P, NVO, D], BF16, tag="vown", bufs=1,
                             name="v_own")
            for st in range(NVO):
                if st == 0:
                    xv_t = xv_t0
                else:
                    xv_t = xvp.tile([P, EO, P], BF16, tag="xv", name=f"xv_t{st}")
                    if st <= 3:
                        # startup-critical: piece-split across both queues so
                        # the first tiles arrive in parallel
                        for e2 in range(0, EO, 2):
                            eng = nc.sync if (st + e2 // 2) % 2 == 0 else nc.gpsimd
                            eng.dma_start(out=xv_t[:, e2:e2 + 2, :],
                                          in_=xv[st, :, e2:e2 + 2, :])
                    else:
                        eng = nc.sync if st % 2 == 0 else nc.gpsimd
                        eng.dma_start(out=xv_t, in_=xv[st])
                ps2 = psB.tile([P, D], F32, tag="psB", name=f"psv{st}")
                # eo-outer: each xv stationary block is loaded once and used
                # for both output halves (the two psum groups are in
                # different banks, so interleaving them is safe)
                for eo in range(EO):
                    for half in range(2):
                        nc.tensor.matmul(
                            ps2[:, half * 512:(half + 1) * 512],
                            lhsT=xv_t[:, eo, :],
                            rhs=wv_sb[:, eo, half * 512:(half + 1) * 512],
                            start=(eo == 0), stop=(eo == EO - 1))
                # evict on DVE: scalar's FIFO is clogged with the blocking
                # K/Q input-DMA issues until ~30us, and DVE is idle until
                # the attention phase anyway
                nc.vector.tensor_copy(out=v_own[:, st, :], in_=ps2)

            # wq/xq (2MB, not consumed until the Q projection ~75us) are
            # gated behind the 6th V eviction: the 25-45us window is where
            # the vin bounces fight for DMA-ring slots and HBM bandwidth,
            # and this keeps 2MB out of it.  Anchoring on st=5 (not the
            # last eviction) leaves scalar's FIFO free for the K-proj
            # evictions that follow right after.
            gate2_sb = const.tile([P, 1], BF16)
            nc.scalar.activation(out=gate2_sb, in_=v_own[:, 5, :1], func=Copy)
            nc.scalar.dma_start(out=wq_sb, in_=wq)
            nc.scalar.dma_start(out=xq_t, in_=xq)

            # bounce copies AFTER the input-DMA stream: HWDGE rings are FIFO
            # per engine, so an eviction-gated bounce emitted mid-loop would
            # block the xv input DMAs queued behind it and starve the PE.
            # Each half-gather fires as soon as its 4 tiles are bounced;
            # gathered output is rank-major == physical s-order, so v_sb is
            # reassembled identically on both cores.  Fill DMAs are gpsimd
            # ONLY: gpsimd issues no later DMAs, so a late collective can't
            # FIFO-block the attention-phase output DMAs on sync/scalar.
            groups = [[2 * p, 2 * p + 1] for p in range(N_CORES // 2)]
            for i in range(2):
                for st in range(NVC):
                    nc.sync.dma_start(out=vins[i][:, st, :],
                                      in_=v_own[:, i * NVC + st, :])
                nc.gpsimd.collective_compute(
                    "AllGather", mybir.AluOpType.bypass, replica_groups=groups,
                    ins=[vins[i].opt()], outs=[vouts[i].opt()])
            for i in range(2):
                for r in range(2):
                    nc.gpsimd.dma_start(
                        out=v_sb[:, r * NVO + i * NVC:r * NVO + (i + 1) * NVC, :],
                        in_=vouts[i][r])

            # K projection -> kt_sb [e-part, k], bias folded in (fp8 DoubleRow)
            for kc in range(4):
                xk_t = xk_ts[kc]
                for do in range(DO):
                    ps = psA.tile([P, 512], F32, tag="psA", name=f"psk{kc}_{do}")
                    for m in range(4):
                        nc.tensor.matmul(
                            ps,
                            lhsT=wk_sb[:, 2 * m:2 * m + 2, do * P:(do + 1) * P],
                            rhs=xk_t[:, 2 * m:2 * m + 2, :],
                            start=(m == 0), stop=(m == 3),
                            perf_mode=mybir.MatmulPerfMode.DoubleRow)
                    nc.scalar.activation(
                        out=kt_sb[:, do, kc * 512:(kc + 1) * 512], in_=ps,
                        func=Ident, bias=bk_sb[:, do:do + 1], scale=1.0 / 16)

            # Q projection -> qt_sb [e-part, q], bias folded in (fp8 DoubleRow)
            for sc in range(2):
                for do in range(DO):
                    ps = psA.tile([P, 512], F32, tag="psA", name=f"psq{sc}_{do}")
                    for m in range(4):
                        nc.tensor.matmul(
                            ps,
                            lhsT=wq_sb[:, 2 * m:2 * m + 2, do * P:(do + 1) * P],
                            rhs=xq_t[:, 2 * m:2 * m + 2, sc * 512:(sc + 1) * 512],
                            start=(m == 0), stop=(m == 3),
                            perf_mode=mybir.MatmulPerfMode.DoubleRow)
                    nc.scalar.activation(
                        out=qt_sb[:, do, sc * 512:(sc + 1) * 512], in_=ps,
                        func=Ident, bias=bq_sb[:, do:do + 1], scale=1.0 / 16)

        # ---------------- attention ----------------
        # scores k-chunk-major: one stationary K-tile streams against ALL
        # active slots' Q columns (slots active for chunk c are j >= 2c,
        # contiguous in qt) - amortizes the fp8 weight loads.  exp lands in
        # pe_all[k-tile][slot*128], then P@V runs slot-major as before.
        with tc.tile_pool(name="pep", bufs=1) as pep, \
             tc.tile_pool(name="recp", bufs=2) as recp, \
             tc.tile_pool(name="outp", bufs=3) as outp, \
             tc.tile_pool(name="psS", bufs=3, space="PSUM") as psS, \
             tc.tile_pool(name="psV", bufs=2, space="PSUM") as psV, \
             tc.tile_pool(name="psD", bufs=1, space="PSUM") as psD:

            pe_all = pep.tile([P, NT, 1024], BF16, name="pe_all")
            dn_ps = psD.tile([P, NQ], F32, tag="dn", name="dn_ps")

            for c in range(4):
                j0 = 2 * c if causal else 0
                w = (NQ - j0) * P
                for i in range(4):
                    t = 4 * c + i
                    dcol = (t // 2 - j0) * P        # diagonal slot's column
                    for p0 in range(0, w, 512):
                        wp = min(512, w - p0)
                        ps = psS.tile([P, wp], F32, tag="s", name=f"ps{t}_{p0}")
                        for m in range(4):
                            nc.tensor.matmul(
                                ps,
                                lhsT=kt_sb[:, 2 * m:2 * m + 2, t * P:(t + 1) * P],
                                rhs=qt_sb[:, 2 * m:2 * m + 2,
                                          j0 * P + p0:j0 * P + p0 + wp],
                                start=(m == 0), stop=(m == 3),
                                perf_mode=mybir.MatmulPerfMode.DoubleRow)
                        if causal and p0 <= dcol < p0 + wp:
                            nc.vector.tensor_tensor(
                                out=ps[:, dcol - p0:dcol - p0 + P],
                                in0=ps[:, dcol - p0:dcol - p0 + P],
                                in1=msk_sb[:, (t % 2) * P:(t % 2 + 1) * P],
                                op=add)
                        nc.scalar.activation(
                            out=pe_all[:, t, j0 * P + p0:j0 * P + p0 + wp],
                            in_=ps, func=Exp, scale=float(1.0 / np.sqrt(D)))

            def av_slot(j):
                n_t = nkt[j]
                av = psV.tile([P, D], F32, tag="av", name=f"av{j}")
                for t in range(n_t):
                    pblk = pe_all[:, t, j * P:(j + 1) * P]
                    nc.tensor.matmul(
                        dn_ps[:, j:j + 1], lhsT=pblk, rhs=ones_sb,
                        start=(t == 0), stop=(t == n_t - 1))
                    for half in range(2):
                        nc.tensor.matmul(
                            av[:, half * 512:(half + 1) * 512],
                            lhsT=pblk,
                            rhs=v_sb[:, t, half * 512:(half + 1) * 512],
                            start=(t == 0), stop=(t == n_t - 1))
                rec = recp.tile([P, 1], F32, tag="rec", name=f"rec{j}")
                nc.vector.reciprocal(out=rec, in_=dn_ps[:, j:j + 1])
                o = outp.tile([P, D], F32, tag="o", name=f"o{j}")
                last = j == (0 if causal else NQ - 1)
                nhf = 4 if last else 2
                hw = D // nhf
                for hf in range(nhf):
                    nc.vector.scalar_tensor_tensor(
                        out=o[:, hf * hw:(hf + 1) * hw],
                        in0=av[:, hf * hw:(hf + 1) * hw], scalar=rec,
                        in1=bv_sb[:, hf * hw:(hf + 1) * hw],
                        op0=mult, op1=add)
                    eng = nc.sync if hf % 2 == 0 else nc.scalar
                    eng.dma_start(out=out[j, :, hf * hw:(hf + 1) * hw],
                                  in_=o[:, hf * hw:(hf + 1) * hw])

            # ascending so early AVs only need early exps and the tail is
            # a single slot's drain chain (rec -> normalize -> out DMA)
            for j in range(NQ):
                av_slot(j)

    nc.compile()
    return nc


def _get_program(causal: bool):
    key = bool(causal)
    if key not in _PROG_CACHE:
        _PROG_CACHE[key] = build_program(key)
    return _PROG_CACHE[key]


def _shard_inputs(encoded_q, encoded_k, encoded_v, W_q, b_q, W_k, b_k,
                  W_v, b_v, causal):
    """Build the per-core in_maps (all host-side numpy, bf16 payloads)."""
    F8 = mybir.dt.np(FP8)
    wqh = np.ascontiguousarray(
        (16.0 * W_q.T).reshape(EO, P, D).transpose(1, 0, 2)).astype(F8)
    wkh = np.ascontiguousarray(
        (16.0 * W_k.T).reshape(EO, P, D).transpose(1, 0, 2)).astype(F8)
    wvh = np.ascontiguousarray(
        W_v.T.reshape(EO, P, D).transpose(1, 0, 2)).astype(BF)
    bqh = np.ascontiguousarray(b_q.reshape(DO, P).T)
    bkh = np.ascontiguousarray(b_k.reshape(DO, P).T)
    bvh = np.ascontiguousarray(np.broadcast_to(b_v, (P, D)))

    ki = np.arange(P)[:, None]
    qi = np.arange(P)[None, :]
    tri = np.where(ki <= qi, 0.0, NEG).astype(np.float32)   # diagonal block
    zer = np.zeros((P, P), np.float32)
    ninf = np.full((P, P), NEG, np.float32)
    # h=0: slot j owns g=2j -> k-tile 2j is diagonal, 2j+1 fully masked
    # h=1: slot j owns g=2j+1 -> k-tile 2j unmasked, 2j+1 diagonal
    mskh = [np.concatenate([tri, ninf], 1), np.concatenate([zer, tri], 1)]

    in_maps = []
    for c in range(N_CORES):
        b, h = divmod(c, 2)
        gts = _slot_gtiles(h, causal)
        Xq = np.concatenate([encoded_q[b, g * P:(g + 1) * P, :] for g in gts], 0)
        xqh = np.ascontiguousarray(
            Xq.T.reshape(EO, P, 1024).transpose(1, 0, 2)).astype(F8)
        xkh = np.ascontiguousarray(
            encoded_k[b].T.reshape(EO, P, 4, 512).transpose(2, 1, 0, 3)).astype(F8)
        # own V half only: parity-h core projects s rows [h*S/2, (h+1)*S/2)
        xvh = np.ascontiguousarray(
            encoded_v[b, h * (S // 2):(h + 1) * (S // 2), :].T
            .reshape(EO, P, NT // 2, P).transpose(2, 1, 0, 3)).astype(BF)
        in_maps.append({
            "xq": xqh, "xk": xkh, "xv": xvh,
            "wq": wqh, "wk": wkh, "wv": wvh,
            "bq": bqh, "bk": bkh, "bv": bvh,
            "msk": mskh[h] if causal else np.zeros((P, 2 * P), np.float32),
        })
    return in_maps


def kernel(encoded_q, encoded_k, encoded_v, W_q, b_q, W_k, b_k, W_v, b_v,
           parameter_mask, _want_trace=False, _trace_dir=None):
    causal = bool(np.asarray(parameter_mask).item())
    encoded_q = np.asarray(encoded_q, np.float32)
    encoded_k = np.asarray(encoded_k, np.float32)
    encoded_v = np.asarray(encoded_v, np.float32)
    nc = _get_program(causal)
    in_maps = _shard_inputs(encoded_q, encoded_k, encoded_v,
                            np.asarray(W_q, np.float32), np.asarray(b_q, np.float32),
                            np.asarray(W_k, np.float32), np.asarray(b_k, np.float32),
                            np.asarray(W_v, np.float32), np.asarray(b_v, np.float32),
                            causal)
    kw = {}
    if _want_trace:
        kw = dict(trace=True, tmpdir=_trace_dir)
    elif not _PROG_CACHE.get(("warm", causal)):
        # first execution pays collective-communicator init (~hundreds of us
        # of skew on one core); absorb it in a throwaway run
        run_bass_kernel_spmd(nc, in_maps, core_ids=list(range(N_CORES)))
        _PROG_CACHE[("warm", causal)] = True
    res = run_bass_kernel_spmd(nc, in_maps, core_ids=list(range(N_CORES)), **kw)

    full = np.empty((B, S, D), np.float32)
    for c in range(N_CORES):
        b, h = divmod(c, 2)
        o = res.results[c]["out"]
        for j, g in enumerate(_slot_gtiles(h, causal)):
            full[b, g * P:(g + 1) * P, :] = o[j]
    if _want_trace:
        return full, res
    return full



# revision 55
# speedup vs baseline: 1.1897x; 1.0225x over previous
"""Trainium2 Bass kernel for single-head causal attention.

  q = Xq @ Wq.T + bq ; k = Xk @ Wk.T + bk ; v = Xv @ Wv.T + bv
  out = softmax((q k^T + causal_mask)/sqrt(D)) @ v

Shapes: B=4, S=2048, D=1024, fp32 in/out.  8 NeuronCores, SPMD.

Sharding: core c handles batch b = c//2, parity h = c%2.  S splits into 16
q-tiles of 128; causal attention for q-tile g touches k-tiles 0..g.  Core
parity h owns q-tiles g = 2j + h (j = 0..7), and slot j statically
processes 2j+2 k-tiles on BOTH parities (identical SPMD program); the
h=0 core's last k-tile per slot is fully masked, so the per-core causal
mask is one static [128, 256] tile covering the last two k-tiles.

V-projection dedupe: the two cores of a batch pair each project only
their OWN half of V (8 s-tiles) and exchange halves through a pairwise
AllGather over DRAM bounce buffers (replica groups [2b, 2b+1]).  The
gathered buffer is rank-major == physical s-order, so v_sb reassembles
identically on both parities with a uniform SPMD program and the math
is bit-identical to projecting both halves locally.  The collective's
~55us doorbell-to-fill latency hides under the K/Q projections and the
scores phase.  (K-proj dedupe doesn't fit: K^T is consumed ~35us after
its half would finish bouncing, well inside the collective latency.)

Compute (all bf16 matmuls, f32 psum):
  - K^T, Q^T projected to [e-part, s] layout, V to [s-part, d]; all
    stay SBUF-resident except the V-half exchange roundtrip.
  - Scores are computed TRANSPOSED ([k, q] blocks): exp output feeds the
    P@V matmul directly as the stationary operand - no PE transposes.
  - softmax denominator = pe-block matmul against a ones column, giving
    [q-part, 1] psum, the right orientation for the final normalize
    (out = av * (1/den) + bv on DVE).
  - attention is software-pipelined one slot deep: scores(j+1) are issued
    before P@V(j) so the exp never stalls the tensor engine.
"""

from contextlib import ExitStack

import ml_dtypes
import numpy as np

import concourse.bacc as bacc
import concourse.mybir as mybir
import concourse.tile as tile
from concourse.bass_utils import run_bass_kernel_spmd

P = 128
D = 1024
S = 2048
B = 4
N_CORES = 8
EO = D // P            # 8 contraction chunks of 128
DO = D // P            # 8 output-dim chunks of 128
NT = S // P            # 16 k/s tiles of 128
NQ = 8                 # q-tile slots per core
F32 = mybir.dt.float32
BF16 = mybir.dt.bfloat16
FP8 = mybir.dt.float8e4
NEG = -1.0e9
BF = ml_dtypes.bfloat16

_PROG_CACHE = {}


def _slot_gtiles(h, causal):
    """q-tile ids (units of 128 rows) owned by parity-h core, slot order."""
    if causal:
        return [2 * j + h for j in range(NQ)]
    return [8 * h + j for j in range(NQ)]


def build_program(causal: bool):
    nc = bacc.Bacc(trn_type="TRN2", target_bir_lowering=False, debug=False)

    def din(name, shape, dt=BF16):
        return nc.dram_tensor(name, shape, dt, kind="ExternalInput").ap()

    xq = din("xq", [P, EO, 1024], FP8)   # Xq^T for this core's 8 q-tiles
    xk = din("xk", [4, P, EO, 512], FP8)  # Xk^T, chunked along s
    # Xv^T for this core's OWN half of the sequence (8 s-tiles): the two
    # cores of a batch pair each project half of V and exchange the halves
    # via a pairwise AllGather (bit-identical to projecting both halves).
    NVO = NT // 2
    xv = din("xv", [NVO, P, EO, P])      # Xv^T own half, blocked [s-tile][e][s]
    wq = din("wq", [P, EO, D], FP8)
    wk = din("wk", [P, EO, D], FP8)
    wv = din("wv", [P, EO, D])
    bq = din("bq", [P, DO], F32)
    bk = din("bk", [P, DO], F32)
    bv = din("bv", [P, D], F32)
    msk = din("msk", [P, 2 * P], F32)    # causal mask for last 2 k-tiles
    out = nc.dram_tensor("out", [NQ, P, D], F32, kind="ExternalOutput").ap()

    Ident = mybir.ActivationFunctionType.Identity
    Copy = mybir.ActivationFunctionType.Copy
    Exp = mybir.ActivationFunctionType.Exp
    add = mybir.AluOpType.add
    mult = mybir.AluOpType.mult

    # slot j processes nkt[j] k-tiles - identical on every core
    nkt = [2 * j + 2 if causal else NT for j in range(NQ)]

    with tile.TileContext(nc, pool_alloc_mode="queue") as tc, ExitStack() as top:
        const = top.enter_context(tc.tile_pool(name="const", bufs=1))
        bq_sb = const.tile([P, DO], F32)
        bk_sb = const.tile([P, DO], F32)
        bv_sb = const.tile([P, D], F32)
        msk_sb = const.tile([P, 2 * P], F32)
        ones_sb = const.tile([P, 1], BF16)
        nc.gpsimd.memset(ones_sb, 1.0)




        # resident projected tensors
        res = top.enter_context(tc.tile_pool(name="res", bufs=1))
        kt_sb = res.tile([P, DO, S], FP8, name="kt_sb")      # K^T [e, k]
        qt_sb = res.tile([P, DO, 1024], FP8, name="qt_sb")   # Q^T [e, q]
        v_sb = res.tile([P, NT, D], BF16, name="v_sb")       # V [s, d] blocked

        # DRAM bounce buffers for the pairwise V-half AllGather, split into
        # two 1MB collectives: the first (own s-tiles 0-3) fires ~14us
        # earlier than a single 2MB gather would complete, halving the
        # time-to-first-V-data and the exposure to collective-latency noise
        ccp = top.enter_context(tc.tile_pool(name="ccp", bufs=1, space="DRAM"))
        # asymmetric [6,2] split: the first gather covers physical s-tiles
        # 0-5 (and 8-13), which is what P@V slots 0-3 consume first - its
        # fill lands ~15us before they need it; the tiny second gather
        # (tiles 6-7 / 14-15) drains quickly behind it
        NVCS = [6, 2]
        NVC0 = [0, 6]
        vins = [ccp.tile([P, NVCS[i], D], BF16, name=f"vin{i}")
                for i in range(2)]
        vouts = [ccp.tile([2, P, NVCS[i], D], BF16, name=f"vout{i}")
                 for i in range(2)]

        # ---------------- projections ----------------
        with tc.tile_pool(name="wt", bufs=2) as wtp, \
             tc.tile_pool(name="xin", bufs=4) as xinp, \
             tc.tile_pool(name="xqp", bufs=1) as xqp, \
             tc.tile_pool(name="xvp", bufs=6) as xvp, \
             tc.tile_pool(name="psA", bufs=3, space="PSUM") as psA, \
             tc.tile_pool(name="psB", bufs=2, space="PSUM") as psB:

            # V projection FIRST: bf16 (1 cyc/row) gives the fp8 K/Q
            # inputs the whole phase to land, so the (2x faster, DMA-hungry)
            # DoubleRow projections never starve.  The startup-critical first
            # chunks (wv/xv eo=0) get their own small descriptors so the
            # first matmul isn't gated on a megabyte landing.
            # Queue discipline: big blocking K/Q input issues live ONLY on
            # scalar (whose first compute need - the K evictions - comes
            # late); sync and gpsimd carry the V-phase inputs + bounces so
            # nothing FIFO-blocks the eviction->bounce->collective chain.
            # The collective can only fire once the WHOLE own-V half is
            # projected, and that needs all of wv - so wv/xv get absolute
            # DMA priority on ALL THREE queues; the K/Q inputs queue up
            # behind them (their compute starts ~25us later).
            wv_sb = wtp.tile([P, EO, D], BF16, tag="wt", name="wv_sb")
            xv_t0 = xvp.tile([P, EO, P], BF16, tag="xv", name="xv_t0")
            qs = [nc.sync, nc.gpsimd, nc.scalar]
            for eo in range(EO):
                qs[eo % 3].dma_start(out=wv_sb[:, eo, :], in_=wv[:, eo, :])
                qs[(eo + 1) % 3].dma_start(out=xv_t0[:, eo, :], in_=xv[0, :, eo, :])
            # consts are needed only at the first K-proj eviction (~45us);
            # park them on scalar behind the K/Q inputs so sync/gpsimd can
            # keep feeding the V phase
            nc.scalar.dma_start(out=bk_sb, in_=bk)
            nc.scalar.dma_start(out=bq_sb, in_=bq)
            nc.scalar.dma_start(out=msk_sb, in_=msk)
            nc.scalar.dma_start(out=bv_sb, in_=bv)
            # Gate the K/Q input wave behind the V working set's arrival:
            # this tiny copy reads the last xv_t0 chunk, so the big scalar
            # DMAs behind it in FIFO order can't steal HBM bandwidth from
            # wv/xv during the V phase's startup-critical first ~15us.
            gate_sb = const.tile([P, 1], BF16)
            nc.scalar.activation(out=gate_sb, in_=xv_t0[:, 7, :1], func=Copy)
            # K/Q inputs: single big transfers on scalar (whose first compute
            # need - the K evictions - comes only after the V phase)
            wk_sb = wtp.tile([P, EO, D], FP8, tag="wt", name="wk_sb")
            nc.scalar.dma_start(out=wk_sb, in_=wk)
            xk_ts = []
            for kc in range(4):
                xk_t = xinp.tile([P, EO, 512], FP8, tag="xin", name=f"xk_t{kc}")
                xk_ts.append(xk_t)
                nc.scalar.dma_start(out=xk_t, in_=xk[kc])
            wq_sb = wtp.tile([P, EO, D], FP8, tag="wt", name="wq_sb")
            xq_t = xqp.tile([P, EO, 1024], FP8, name="xq_t")

            v_own = xvp.tile([P, NVO, D], BF16, tag="vown", bufs=1,
                             name="v_own")
            for st in range(NVO):
                if st == 0:
                    xv_t = xv_t0
                else:
                    xv_t = xvp.tile([P, EO, P], BF16, tag="xv", name=f"xv_t{st}")
                    if st <= 3:
                        # startup-critical: piece-split across both queues so
                        # the first tiles arrive in parallel
                        for e2 in range(0, EO, 2):
                            eng = nc.sync if (st + e2 // 2) % 2 == 0 else nc.gpsimd
                            eng.dma_start(out=xv_t[:, e2:e2 + 2, :],
                                          in_=xv[st, :, e2:e2 + 2, :])
                    else:
                        eng = nc.sync if st % 2 == 0 else nc.gpsimd
                        eng.dma_start(out=xv_t, in_=xv[st])
                ps2 = psB.tile([P, D], F32, tag="psB", name=f"psv{st}")
                # eo-outer: each xv stationary block is loaded once and used
                # for both output halves (the two psum groups are in
                # different banks, so interleaving them is safe)
                for eo in range(EO):
                    for half in range(2):
                        nc.tensor.matmul(
                            ps2[:, half * 512:(half + 1) * 512],
                            lhsT=xv_t[:, eo, :],
                            rhs=wv_sb[:, eo, half * 512:(half + 1) * 512],
                            start=(eo == 0), stop=(eo == EO - 1))
                # evict on DVE: scalar's FIFO is clogged with the blocking
                # K/Q input-DMA issues until ~30us, and DVE is idle until
                # the attention phase anyway
                nc.vector.tensor_copy(out=v_own[:, st, :], in_=ps2)

            # wq/xq (2MB, not consumed until the Q projection ~75us) are
            # gated behind the 6th V eviction: the 25-45us window is where
            # the vin bounces fight for DMA-ring slots and HBM bandwidth,
            # and this keeps 2MB out of it.  Anchoring on st=5 (not the
            # last eviction) leaves scalar's FIFO free for the K-proj
            # evictions that follow right after.
            gate2_sb = const.tile([P, 1], BF16)
            nc.scalar.activation(out=gate2_sb, in_=v_own[:, 5, :1], func=Copy)
            nc.scalar.dma_start(out=wq_sb, in_=wq)
            nc.scalar.dma_start(out=xq_t, in_=xq)

            # bounce copies AFTER the input-DMA stream: HWDGE rings are FIFO
            # per engine, so an eviction-gated bounce emitted mid-loop would
            # block the xv input DMAs queued behind it and starve the PE.
            # Each half-gather fires as soon as its 4 tiles are bounced;
            # gathered output is rank-major == physical s-order, so v_sb is
            # reassembled identically on both cores.  Fill DMAs are gpsimd
            # ONLY: gpsimd issues no later DMAs, so a late collective can't
            # FIFO-block the attention-phase output DMAs on sync/scalar.
            groups = [[2 * p, 2 * p + 1] for p in range(N_CORES // 2)]
            for i in range(2):
                for st in range(NVCS[i]):
                    nc.sync.dma_start(out=vins[i][:, st, :],
                                      in_=v_own[:, NVC0[i] + st, :])
                nc.gpsimd.collective_compute(
                    "AllGather", mybir.AluOpType.bypass, replica_groups=groups,
                    ins=[vins[i].opt()], outs=[vouts[i].opt()])
            for i in range(2):
                for r in range(2):
                    nc.gpsimd.dma_start(
                        out=v_sb[:, r * NVO + NVC0[i]:
                                 r * NVO + NVC0[i] + NVCS[i], :],
                        in_=vouts[i][r])

            # K projection -> kt_sb [e-part, k], bias folded in (fp8 DoubleRow)
            for kc in range(4):
                xk_t = xk_ts[kc]
                for do in range(DO):
                    ps = psA.tile([P, 512], F32, tag="psA", name=f"psk{kc}_{do}")
                    for m in range(4):
                        nc.tensor.matmul(
                            ps,
                            lhsT=wk_sb[:, 2 * m:2 * m + 2, do * P:(do + 1) * P],
                            rhs=xk_t[:, 2 * m:2 * m + 2, :],
                            start=(m == 0), stop=(m == 3),
                            perf_mode=mybir.MatmulPerfMode.DoubleRow)
                    nc.scalar.activation(
                        out=kt_sb[:, do, kc * 512:(kc + 1) * 512], in_=ps,
                        func=Ident, bias=bk_sb[:, do:do + 1], scale=1.0 / 16)

            # Q projection -> qt_sb [e-part, q], bias folded in (fp8 DoubleRow)
            for sc in range(2):
                for do in range(DO):
                    ps = psA.tile([P, 512], F32, tag="psA", name=f"psq{sc}_{do}")
                    for m in range(4):
                        nc.tensor.matmul(
                            ps,
                            lhsT=wq_sb[:, 2 * m:2 * m + 2, do * P:(do + 1) * P],
                            rhs=xq_t[:, 2 * m:2 * m + 2, sc * 512:(sc + 1) * 512],
                            start=(m == 0), stop=(m == 3),
                            perf_mode=mybir.MatmulPerfMode.DoubleRow)
                    nc.scalar.activation(
                        out=qt_sb[:, do, sc * 512:(sc + 1) * 512], in_=ps,
                        func=Ident, bias=bq_sb[:, do:do + 1], scale=1.0 / 16)

        # ---------------- attention ----------------
        # scores k-chunk-major: one stationary K-tile streams against ALL
        # active slots' Q columns (slots active for chunk c are j >= 2c,
        # contiguous in qt) - amortizes the fp8 weight loads.  exp lands in
        # pe_all[k-tile][slot*128], then P@V runs slot-major as before.
        with tc.tile_pool(name="pep", bufs=1) as pep, \
             tc.tile_pool(name="recp", bufs=2) as recp, \
             tc.tile_pool(name="outp", bufs=3) as outp, \
             tc.tile_pool(name="psS", bufs=3, space="PSUM") as psS, \
             tc.tile_pool(name="psV", bufs=2, space="PSUM") as psV, \
             tc.tile_pool(name="psD", bufs=1, space="PSUM") as psD:

            pe_all = pep.tile([P, NT, 1024], BF16, name="pe_all")
            dn_ps = psD.tile([P, NQ], F32, tag="dn", name="dn_ps")

            for c in range(4):
                j0 = 2 * c if causal else 0
                w = (NQ - j0) * P
                for i in range(4):
                    t = 4 * c + i
                    dcol = (t // 2 - j0) * P        # diagonal slot's column
                    for p0 in range(0, w, 512):
                        wp = min(512, w - p0)
                        ps = psS.tile([P, wp], F32, tag="s", name=f"ps{t}_{p0}")
                        for m in range(4):
                            nc.tensor.matmul(
                                ps,
                                lhsT=kt_sb[:, 2 * m:2 * m + 2, t * P:(t + 1) * P],
                                rhs=qt_sb[:, 2 * m:2 * m + 2,
                                          j0 * P + p0:j0 * P + p0 + wp],
                                start=(m == 0), stop=(m == 3),
                                perf_mode=mybir.MatmulPerfMode.DoubleRow)
                        if causal and p0 <= dcol < p0 + wp:
                            nc.vector.tensor_tensor(
                                out=ps[:, dcol - p0:dcol - p0 + P],
                                in0=ps[:, dcol - p0:dcol - p0 + P],
                                in1=msk_sb[:, (t % 2) * P:(t % 2 + 1) * P],
                                op=add)
                        nc.scalar.activation(
                            out=pe_all[:, t, j0 * P + p0:j0 * P + p0 + wp],
                            in_=ps, func=Exp, scale=float(1.0 / np.sqrt(D)))

            def av_slot(j):
                n_t = nkt[j]
                av = psV.tile([P, D], F32, tag="av", name=f"av{j}")
                for t in range(n_t):
                    pblk = pe_all[:, t, j * P:(j + 1) * P]
                    nc.tensor.matmul(
                        dn_ps[:, j:j + 1], lhsT=pblk, rhs=ones_sb,
                        start=(t == 0), stop=(t == n_t - 1))
                    for half in range(2):
                        nc.tensor.matmul(
                            av[:, half * 512:(half + 1) * 512],
                            lhsT=pblk,
                            rhs=v_sb[:, t, half * 512:(half + 1) * 512],
                            start=(t == 0), stop=(t == n_t - 1))
                rec = recp.tile([P, 1], F32, tag="rec", name=f"rec{j}")
                nc.vector.reciprocal(out=rec, in_=dn_ps[:, j:j + 1])
                o = outp.tile([P, D], F32, tag="o", name=f"o{j}")
                last = j == (0 if causal else NQ - 1)
                nhf = 4 if last else 2
                hw = D // nhf
                for hf in range(nhf):
                    nc.vector.scalar_tensor_tensor(
                        out=o[:, hf * hw:(hf + 1) * hw],
                        in0=av[:, hf * hw:(hf + 1) * hw], scalar=rec,
                        in1=bv_sb[:, hf * hw:(hf + 1) * hw],
                        op0=mult, op1=add)
                    eng = nc.sync if hf % 2 == 0 else nc.scalar
                    eng.dma_start(out=out[j, :, hf * hw:(hf + 1) * hw],
                                  in_=o[:, hf * hw:(hf + 1) * hw])

            # ascending so early AVs only need early exps and the tail is
            # a single slot's drain chain (rec -> normalize -> out DMA)
            for j in range(NQ):
                av_slot(j)

    nc.compile()
    return nc


def _get_program(causal: bool):
    key = bool(causal)
    if key not in _PROG_CACHE:
        _PROG_CACHE[key] = build_program(key)
    return _PROG_CACHE[key]


def _shard_inputs(encoded_q, encoded_k, encoded_v, W_q, b_q, W_k, b_k,
                  W_v, b_v, causal):
    """Build the per-core in_maps (all host-side numpy, bf16 payloads)."""
    F8 = mybir.dt.np(FP8)
    wqh = np.ascontiguousarray(
        (16.0 * W_q.T).reshape(EO, P, D).transpose(1, 0, 2)).astype(F8)
    wkh = np.ascontiguousarray(
        (16.0 * W_k.T).reshape(EO, P, D).transpose(1, 0, 2)).astype(F8)
    wvh = np.ascontiguousarray(
        W_v.T.reshape(EO, P, D).transpose(1, 0, 2)).astype(BF)
    bqh = np.ascontiguousarray(b_q.reshape(DO, P).T)
    bkh = np.ascontiguousarray(b_k.reshape(DO, P).T)
    bvh = np.ascontiguousarray(np.broadcast_to(b_v, (P, D)))

    ki = np.arange(P)[:, None]
    qi = np.arange(P)[None, :]
    tri = np.where(ki <= qi, 0.0, NEG).astype(np.float32)   # diagonal block
    zer = np.zeros((P, P), np.float32)
    ninf = np.full((P, P), NEG, np.float32)
    # h=0: slot j owns g=2j -> k-tile 2j is diagonal, 2j+1 fully masked
    # h=1: slot j owns g=2j+1 -> k-tile 2j unmasked, 2j+1 diagonal
    mskh = [np.concatenate([tri, ninf], 1), np.concatenate([zer, tri], 1)]

    in_maps = []
    for c in range(N_CORES):
        b, h = divmod(c, 2)
        gts = _slot_gtiles(h, causal)
        Xq = np.concatenate([encoded_q[b, g * P:(g + 1) * P, :] for g in gts], 0)
        xqh = np.ascontiguousarray(
            Xq.T.reshape(EO, P, 1024).transpose(1, 0, 2)).astype(F8)
        xkh = np.ascontiguousarray(
            encoded_k[b].T.reshape(EO, P, 4, 512).transpose(2, 1, 0, 3)).astype(F8)
        # own V half only: parity-h core projects s rows [h*S/2, (h+1)*S/2)
        xvh = np.ascontiguousarray(
            encoded_v[b, h * (S // 2):(h + 1) * (S // 2), :].T
            .reshape(EO, P, NT // 2, P).transpose(2, 1, 0, 3)).astype(BF)
        in_maps.append({
            "xq": xqh, "xk": xkh, "xv": xvh,
            "wq": wqh, "wk": wkh, "wv": wvh,
            "bq": bqh, "bk": bkh, "bv": bvh,
            "msk": mskh[h] if causal else np.zeros((P, 2 * P), np.float32),
        })
    return in_maps


def kernel(encoded_q, encoded_k, encoded_v, W_q, b_q, W_k, b_k, W_v, b_v,
           parameter_mask, _want_trace=False, _trace_dir=None):
    causal = bool(np.asarray(parameter_mask).item())
    encoded_q = np.asarray(encoded_q, np.float32)
    encoded_k = np.asarray(encoded_k, np.float32)
    encoded_v = np.asarray(encoded_v, np.float32)
    nc = _get_program(causal)
    in_maps = _shard_inputs(encoded_q, encoded_k, encoded_v,
                            np.asarray(W_q, np.float32), np.asarray(b_q, np.float32),
                            np.asarray(W_k, np.float32), np.asarray(b_k, np.float32),
                            np.asarray(W_v, np.float32), np.asarray(b_v, np.float32),
                            causal)
    kw = {}
    if _want_trace:
        kw = dict(trace=True, tmpdir=_trace_dir)
    elif not _PROG_CACHE.get(("warm", causal)):
        # first execution pays collective-communicator init (~hundreds of us
        # of skew on one core); absorb it in a throwaway run
        run_bass_kernel_spmd(nc, in_maps, core_ids=list(range(N_CORES)))
        _PROG_CACHE[("warm", causal)] = True
    res = run_bass_kernel_spmd(nc, in_maps, core_ids=list(range(N_CORES)), **kw)

    full = np.empty((B, S, D), np.float32)
    for c in range(N_CORES):
        b, h = divmod(c, 2)
        o = res.results[c]["out"]
        for j, g in enumerate(_slot_gtiles(h, causal)):
            full[b, g * P:(g + 1) * P, :] = o[j]
    if _want_trace:
        return full, res
    return full

